# revision 1
# baseline (speedup 1.0000x reference)
"""Luong attention Trainium2 Bass kernel.

Computes, per batch b:
    q = query @ W^T + bias                  [B,Tq,H]
    scores = q @ keys^T                     [B,Tq,Tk]
    attn = softmax(scores, axis=-1)
    context = attn @ keys                   [B,Tq,H]
returns (context, attn).

Sharding: batch dim B=32 split across 8 NeuronCores (4 each); W/bias replicated.

Per-core dataflow (per batch, Tq=Tk=1024, H=512, partitions P=128):
  - query tiles [128t,512h] are PE-transposed to qT [h_part, t] (the linear and
    the scores matmuls both contract over h, which must sit on partitions).
  - keys tiles are loaded twice-in-spirit: natural [s_part, h] (context rhs)
    and PE-transposed keysT [h_part, s] (scores rhs).
  - linear: qlinT[o_part, t] = W^T-chunks.T @ qT chunks, bias fused into the
    PSUM->SBUF copy (per-partition scalar add).
  - scores psum [t_part, s] -> softmax along free dim: reduce_max(negate) then
    ACT Exp with bias=-max and accum_out giving the row sums in the same pass.
  - unnormalized exp tiles are PE-transposed to expT [s_part, t] = lhsT of the
    context matmul; 1/sum folds into the context PSUM->SBUF copy (scalar mul
    along t partitions) and into the attn output copy.
"""

import numpy as np

import concourse.bass as bass
import concourse.mybir as mybir
from concourse import bacc
from concourse import bass_utils
from concourse.tile import TileContext
from concourse.masks import make_identity

B, TQ, TK, H = 32, 1024, 1024, 512
N_CORES = 8
B_LOC = B // N_CORES
P = 128
N_HJ = H // P        # h chunks of 128
N_TI = TQ // P       # t tiles of 128
N_SI = TK // P       # s tiles of 128
NMM = 512            # moving-operand width per matmul
F32 = mybir.dt.float32
F32R = mybir.dt.float32r

# Matmul input dtype knob: F32 (safe, 4 cyc/row) or F32R (tf32-like, 1 cyc/row)
MM_DT = F32


def build_nc(b_loc: int = B_LOC, mm_dt=MM_DT):
    nc = bacc.Bacc("TRN2", target_bir_lowering=False)
    q_in = nc.dram_tensor("query", [b_loc, TQ, H], F32, kind="ExternalInput")
    k_in = nc.dram_tensor("keys", [b_loc, TK, H], F32, kind="ExternalInput")
    w_in = nc.dram_tensor("W", [H, H], F32, kind="ExternalInput")
    bias_in = nc.dram_tensor("b", [H], F32, kind="ExternalInput")
    ctx_out = nc.dram_tensor("context", [b_loc, TQ, H], F32, kind="ExternalOutput")
    attn_out = nc.dram_tensor("attn", [b_loc, TQ, TK], F32, kind="ExternalOutput")

    rounds_for_mm = mm_dt != F32

    with TileContext(nc) as tc:
        with tc.tile_pool(name="consts", bufs=1) as consts, \
             tc.tile_pool(name="big", bufs=1) as big, \
             tc.tile_pool(name="work", bufs=3) as work, \
             tc.tile_pool(name="small", bufs=6) as small, \
             tc.tile_pool(name="pst", bufs=2, space="PSUM") as pst, \
             tc.tile_pool(name="psmm", bufs=2, space="PSUM") as psmm:

            identity = consts.tile([P, P], F32)
            make_identity(nc, identity)

            bias_sb = consts.tile([P, N_HJ], F32)
            nc.sync.dma_start(out=bias_sb, in_=bias_in[:].rearrange("(a p) -> p a", p=P))

            # W^T chunks: wt[p, hj, o] = W[o, hj*128+p]
            w_nat = consts.tile([P, N_HJ, H], F32)
            nc.sync.dma_start(out=w_nat, in_=w_in[:, :].rearrange("(oi p) h -> p oi h", p=P))
            wt = consts.tile([P, N_HJ, H], mm_dt)
            for oi in range(N_HJ):
                ps_w = pst.tile([P, H], F32, tag="pst")
                for hj in range(N_HJ):
                    nc.tensor.transpose(
                        ps_w[:, hj * P:(hj + 1) * P], w_nat[:, oi, hj * P:(hj + 1) * P], identity)
                nc.vector.tensor_copy(
                    out=wt[:, :, oi * P:(oi + 1) * P],
                    in_=ps_w.rearrange("p (hj t) -> p hj t", hj=N_HJ))

            for bi in range(b_loc):
                # ---- query transpose: qT[p, hj, t] = query[bi, t, hj*128+p] ----
                qT = big.tile([P, N_HJ, TQ], mm_dt, tag="qT")
                for ti in range(N_TI):
                    qn = work.tile([P, H], F32, tag="qn")
                    nc.sync.dma_start(out=qn, in_=q_in[bi, ti * P:(ti + 1) * P, :])
                    ps_q = pst.tile([P, H], F32, tag="pst")
                    for hj in range(N_HJ):
                        nc.tensor.transpose(
                            ps_q[:, hj * P:(hj + 1) * P], qn[:, hj * P:(hj + 1) * P], identity)
                    nc.vector.tensor_copy(
                        out=qT[:, :, ti * P:(ti + 1) * P],
                        in_=ps_q.rearrange("p (hj t) -> p hj t", hj=N_HJ))

                # ---- keys: natural [s_part, h] + transposed keysT[p, hj, s] ----
                keys_nat = big.tile([P, N_SI, H], F32, tag="keys_nat")
                keysT = big.tile([P, N_HJ, TK], mm_dt, tag="keysT")
                if rounds_for_mm:
                    keys_mm = big.tile([P, N_SI, H], mm_dt, tag="keys_mm")
                else:
                    keys_mm = keys_nat
                for si in range(N_SI):
                    nc.sync.dma_start(out=keys_nat[:, si, :], in_=k_in[bi, si * P:(si + 1) * P, :])
                    if rounds_for_mm:
                        nc.gpsimd.tensor_copy(out=keys_mm[:, si, :], in_=keys_nat[:, si, :])
                    ps_k = pst.tile([P, H], F32, tag="pst")
                    for hj in range(N_HJ):
                        nc.tensor.transpose(
                            ps_k[:, hj * P:(hj + 1) * P],
                            keys_nat[:, si, hj * P:(hj + 1) * P], identity)
                    nc.vector.tensor_copy(
                        out=keysT[:, :, si * P:(si + 1) * P],
                        in_=ps_k.rearrange("p (hj t) -> p hj t", hj=N_HJ))

                # ---- linear: qlinT[p, oi, t] = (query @ W^T + b)^T ----
                qlinT = big.tile([P, N_HJ, TQ], mm_dt, tag="qlinT")
                for oi in range(N_HJ):
                    for tn in range(TQ // NMM):
                        ps_l = psmm.tile([P, NMM], F32, tag="mm")
                        for hj in range(N_HJ):
                            nc.tensor.matmul(
                                ps_l,
                                wt[:, hj, oi * P:(oi + 1) * P],
                                qT[:, hj, tn * NMM:(tn + 1) * NMM],
                                start=(hj == 0), stop=(hj == N_HJ - 1))
                        nc.vector.tensor_scalar_add(
                            out=qlinT[:, oi, tn * NMM:(tn + 1) * NMM],
                            in0=ps_l, scalar1=bias_sb[:, oi:oi + 1])

                # ---- attention, per 128-row tile of queries ----
                for ti in range(N_TI):
                    ps_s = psmm.tile([P, TK], F32, tag="score")
                    for sj in range(TK // NMM):
                        for hj in range(N_HJ):
                            nc.tensor.matmul(
                                ps_s[:, sj * NMM:(sj + 1) * NMM],
                                qlinT[:, hj, ti * P:(ti + 1) * P],
                                keysT[:, hj, sj * NMM:(sj + 1) * NMM],
                                start=(hj == 0), stop=(hj == N_HJ - 1))

                    negmax = small.tile([P, 1], F32, tag="negmax")
                    nc.vector.reduce_max(negmax, ps_s, axis=mybir.AxisListType.X, negate=True)
                    exp_sb = work.tile([P, TK], F32, tag="exp")
                    sumexp = small.tile([P, 1], F32, tag="sumexp")
                    nc.scalar.activation(
                        out=exp_sb, in_=ps_s,
                        func=mybir.ActivationFunctionType.Exp,
                        bias=negmax[:, 0:1], scale=1.0,
                        accum_out=sumexp[:, 0:1])
                    recip = small.tile([P, 1], F32, tag="recip")
                    nc.vector.reciprocal(recip[:, 0:1], sumexp[:, 0:1])

                    attn_sb = work.tile([P, TK], F32, tag="attn")
                    nc.gpsimd.tensor_scalar_mul(out=attn_sb, in0=exp_sb, scalar1=recip[:, 0:1])
                    nc.sync.dma_start(out=attn_out[bi, ti * P:(ti + 1) * P, :], in_=attn_sb)

                    # expT[p, sj, t] = exp[t, sj*128+p]  (unnormalized)
                    expT = work.tile([P, N_SI, P], mm_dt, tag="expT")
                    for half in range(2):
                        ps_e = pst.tile([P, NMM], F32, tag="pst")
                        for k4 in range(4):
                            sj = half * 4 + k4
                            nc.tensor.transpose(
                                ps_e[:, k4 * P:(k4 + 1) * P],
                                exp_sb[:, sj * P:(sj + 1) * P], identity)
                        nc.vector.tensor_copy(
                            out=expT[:, half * 4:(half + 1) * 4, :],
                            in_=ps_e.rearrange("p (k t) -> p k t", k=4))

                    ps_c = psmm.tile([P, H], F32, tag="mm")
                    for sj in range(N_SI):
                        nc.tensor.matmul(
                            ps_c, expT[:, sj, :], keys_mm[:, sj, :],
                            start=(sj == 0), stop=(sj == N_SI - 1))
                    ctx_sb = work.tile([P, H], F32, tag="ctx")
                    nc.vector.tensor_scalar_mul(out=ctx_sb, in0=ps_c, scalar1=recip[:, 0:1])
                    nc.sync.dma_start(out=ctx_out[bi, ti * P:(ti + 1) * P, :], in_=ctx_sb)

    nc.finalize()
    return nc


_NC_CACHE = {}


def _get_nc():
    key = (B_LOC, MM_DT)
    if key not in _NC_CACHE:
        _NC_CACHE[key] = build_nc(B_LOC, MM_DT)
    return _NC_CACHE[key]


def run_sharded(query, keys, W, b, trace=False):
    """Run on 8 cores, batch-sharded. Returns (context, attn, BassKernelResults)."""
    query = np.ascontiguousarray(query, dtype=np.float32)
    keys = np.ascontiguousarray(keys, dtype=np.float32)
    W = np.ascontiguousarray(W, dtype=np.float32)
    b = np.ascontiguousarray(b, dtype=np.float32)

    nc = _get_nc()
    in_maps = [
        {
            "query": query[c * B_LOC:(c + 1) * B_LOC],
            "keys": keys[c * B_LOC:(c + 1) * B_LOC],
            "W": W,
            "b": b,
        }
        for c in range(N_CORES)
    ]
    res = bass_utils.run_bass_kernel_spmd(
        nc, in_maps, core_ids=list(range(N_CORES)), trace=trace)
    context = np.concatenate([res.results[c]["context"] for c in range(N_CORES)], axis=0)
    attn = np.concatenate([res.results[c]["attn"] for c in range(N_CORES)], axis=0)
    return context, attn, res


def kernel(query, keys, W, b):
    context, attn, _ = run_sharded(query, keys, W, b, trace=False)
    return context, attn
